# revision 16
# baseline (speedup 1.0000x reference)
"""CARAFE content-aware upsampling on 8 Trainium2 NeuronCores (Bass/Tile).

Problem: x[2,256,64,64], 1x1 compress conv (256->32), 5x5 encoder conv
(32->100), pixel-shuffle(r=2) + softmax over 25 taps, then dynamic-filter
reassembly: out[b,c,2h+r1,2w+r2] = sum_k x[b,c,h+di,w+dj] * softmax_w.

Sharding: pure data-parallel over (batch, 16-row H slices) -> 8 cores.

Per-core mapping (the cost model charges ~630ns of serialized HWDGE per
DMA and ~1.1us of Pool time per SWDGE DMA, so DMA instruction count is
minimized and all mid-kernel data movement uses engine copies):
  - Host prep ships x twice: channel-major (f32r) for the compress conv
    and window-major xcall [120, 8192] (bf16) holding the overlapping
    6x20 MAC stationaries, so no on-device transpose/gather.
  - compress conv (f32r) writes a 4-high column-shifted stack of y1
    (cross-partition psum copies), so the 5x5 encoder conv contracts 4
    dj taps per K=128 matmul (plus a K=32 dj=4 pass): 10 matmuls per
    row parity instead of 25. Encoder output channels are laid out
    o'' = sub*32 + tap on 128 partitions.
  - softmax stays channel-major (select-matrix matmuls for tap sums and
    reciprocal broadcast); the gating ro=1 parity is split into two
    wi-halves that pipeline through Act/PE/DVE.
  - The normalized weights are relaid out to [25, 2048] (taps on
    partitions) by cross-partition DVE copies, then the block-sparse
    band matrix ybig [120, 4096] (bf16) is built by 128 tiny PE matmuls
    against host-prepared 0/1 placement matrices - the matmul writes the
    band's zeros too, so no memset and no scatter DMAs.
  - The 25-tap dynamic-filter sum runs on PE as 64 bf16 [120]x[128]
    matmuls (stationary = xcall windows, moving = band views).
  - Output is stored bf16 (8 DMAs) and upcast on host; a chain of dummy
    matmuls at t=0 ramps the PE p-state before real work arrives.
"""

import sys

sys.path.insert(0, "/opt/trn_rl_repo")

import numpy as np
import ml_dtypes

import concourse.bacc as bacc
import concourse.bass as bass
import concourse.tile as tile
from concourse import mybir
from concourse.ap import AP

F32 = mybir.dt.float32
F32R = mybir.dt.float32r
BF16 = mybir.dt.bfloat16
BF16NP = ml_dtypes.bfloat16

# geometry
B, C, H, W = 2, 256, 64, 64
RATIO, K_UP, C_MID, ENC_K = 2, 5, 32, 5
NK = RATIO * RATIO * K_UP * K_UP  # 100
HSLICE = 16                       # output source rows per core
ROWS = HSLICE + 4                 # with 2-row halo each side
WP = W + 4                        # padded width
PADPOS = ROWS * WP                # 1360
NCORES = 8
KDIM = 120                        # 6x20 window pixels per row-pair block
YF = 4096                         # band matrix columns
NPRIME = 34                       # PE p-state priming matmuls


def build_program(with_ebias: bool):
    nc = bacc.Bacc()
    xin_d = nc.declare_dram_parameter("xin", [2, 128, PADPOS + 1], BF16, isOutput=False)
    xc_d = nc.declare_dram_parameter("xcall", [KDIM, 8192], BF16, isOutput=False)
    wp_d = nc.declare_dram_parameter("wp128", [128, 64], BF16, isOutput=False)
    wet_d = nc.declare_dram_parameter("wenc", [128, 1280], BF16, isOutput=False)
    selb_d = nc.declare_dram_parameter("selb", [128, 4], BF16, isOutput=False)
    selt_d = nc.declare_dram_parameter("selt", [4, 128], F32R, isOutput=False)
    pp_d = nc.declare_dram_parameter("ppack", [25, 32 * KDIM], BF16, isOutput=False)
    if with_ebias:
        ebias_d = nc.declare_dram_parameter("ebias", [2, 128, 512], F32, isOutput=False)
    out_d = nc.declare_dram_parameter("out", [2, 128, YF], BF16, isOutput=True)

    with tile.TileContext(nc) as tc:
        # Partition-crossing DMA APs (relayout) confuse the byte-range race
        # detector; deps are tracked at tensor granularity regardless.
        tc.race_detector_enabled = False
        # PSUM is 8 banks x 2KB/partition; pools cost bufs x (bank-rounded
        # slot per tag). psC/psE are scoped to the conv/softmax phase and
        # released before the MAC pool opens: 4+2+2 banks early, 4+4 late.
        with (
            tc.tile_pool(name="persist", bufs=1) as pp,
            tc.tile_pool(name="psS", bufs=4, space="PSUM") as psS,   # prime/band
        ):
            # ---- PE p-state priming: keep PE busy from t=0 so real matmuls
            # run at the full-ramp cycle time when inputs arrive.
            dummy = pp.tile([128, 128], BF16, tag="dummy")
            nc.vector.memset(dummy[:], 0.0)
            for _ in range(NPRIME):
                ps = psS.tile([128, 512], F32, tag="band")
                nc.tensor.matmul(
                    ps[:, 0:128], dummy[:], dummy[:], start=True, stop=True
                )

            # ---- input loads ----
            # All loads ride HWDGE queues (SP/Act/DVE) so the Pool engine
            # stays free to serve as a third copy engine. Coalesced into one
            # DMA per tensor; wire order tracks need-order.
            wp128 = pp.tile([128, 64], BF16, tag="wp128")
            nc.scalar.dma_start(wp128[:], wp_d[:])
            xin0 = pp.tile([128, PADPOS + 1], BF16, tag="xin0")
            xin1 = pp.tile([128, PADPOS + 1], BF16, tag="xin1")
            chunks = [(0, 340), (340, 340), (680, 340), (1020, PADPOS - 1020)]
            nc.sync.dma_start(xin0[:], xin_d[0])
            nc.sync.dma_start(xin1[:], xin_d[1])
            wenc = pp.tile([128, 1280], BF16, tag="wenc")
            nc.scalar.dma_start(wenc[:], wet_d[:])
            selb = pp.tile([128, 4], BF16, tag="selb")
            nc.scalar.dma_start(selb[:], selb_d[:])
            selt = pp.tile([4, 128], F32R, tag="selt")
            nc.scalar.dma_start(selt[:], selt_d[:])
            ppk = pp.tile([25, 32 * KDIM], BF16, tag="ppack")
            nc.scalar.dma_start(ppk[:], pp_d[:])
            if with_ebias:
                ebias = []
                for ro in range(2):
                    t = pp.tile([128, 512], F32, name=f"ebias{ro}", tag=f"ebias{ro}")
                    nc.scalar.dma_start(t[:], ebias_d[ro])
                    ebias.append(t)
            xcall = pp.tile([KDIM, 8192], BF16, tag="xcall")
            nc.sync.dma_start(xcall[:], xc_d[:])

            # ---- compress conv -> stacked [128, PADPOS]: row block 32b
            # holds y1[m, p+b]. Blocks 0/1 come from matmul pairs against
            # col-shifted x; blocks 2/3 are chunk-aligned shifted copies of
            # blocks 0/1, so the encoder contracts four dj taps per matmul
            # with K=128 at base partition 0.
            stk = pp.tile([128, PADPOS], BF16, tag="stk")
            ctx_inner = tc.tile_pool(name="psC", bufs=2, space="PSUM")
            psC = ctx_inner.__enter__()
            ctx_enc = tc.tile_pool(name="psE", bufs=2, space="PSUM")
            psE = ctx_enc.__enter__()
            for ci, (off, n) in enumerate(chunks):
                for b in range(2):
                    ps = psC.tile([128, 512], F32, tag="c")
                    nc.tensor.matmul(
                        ps[0:C_MID, :n],
                        wp128[:, 0:32], xin0[:, off + b:off + b + n],
                        start=True, stop=False,
                    )
                    nc.tensor.matmul(
                        ps[0:C_MID, :n],
                        wp128[:, 32:64], xin1[:, off + b:off + b + n],
                        start=False, stop=True,
                    )
                    # cross-partition copy drops the shifted rows into the
                    # stack's b-th 32-row block (PSUM source: only DVE/Act
                    # may read PSUM; GPSIMD cannot).
                    eng = (nc.vector.tensor_copy, nc.scalar.copy)[b]
                    eng(stk[32 * b:32 * b + 32, off:off + n], ps[0:C_MID, :n])
                # blocks 2/3: +2-shifted copies of blocks 0/1, chunk-aligned
                # (chunk i's source columns live in chunks i and i+1, so
                # shift the window 2 left to stay within loaded data)
                s0 = max(0, off - 2)
                s1 = off + n - 2
                # SBUF->SBUF bf16: Pool is free (no SWDGE anymore), keep
                # DVE/Act clear for the PSUM-source copies
                nc.gpsimd.tensor_copy(stk[64:128, s0:s1], stk[0:64, s0 + 2:s1 + 2])
                if ci == len(chunks) - 1:
                    # last two columns of blocks 2/3 (never read, but keep
                    # them initialized for the simulator)
                    nc.gpsimd.tensor_copy(
                        stk[64:128, s1:s1 + 2], stk[0:64, s1:s1 + 2]
                    )

            # ---- encoder conv + softmax ----
            # ro0's softmax chain is hidden under ro1's encoder; ro1's chain
            # gates the MAC, so it is split into two wi-halves that pipeline
            # through PE/Act/DVE, letting its band build start ~2.5us earlier.
            yM = []
            yMp = []

            def softmax_norm(psenc, cols, names):
                # returns normalized bf16 weights tile [128, cols]
                y2e = pp.tile([128, cols], BF16, name=names + "e", tag=names + "e")
                if with_ebias:
                    y2f = pp.tile([128, cols], F32, name=names + "f", tag=names + "f")
                    nc.vector.scalar_tensor_tensor(
                        y2f[:], psenc, 1.0, ebias_sl[names],
                        op0=mybir.AluOpType.mult, op1=mybir.AluOpType.add,
                    )
                    nc.scalar.activation(
                        y2e[:], y2f[:], mybir.ActivationFunctionType.Exp
                    )
                else:
                    nc.scalar.activation(
                        y2e[:], psenc, mybir.ActivationFunctionType.Exp
                    )
                pss = psC.tile([128, 512], F32, tag="c")
                nc.tensor.matmul(
                    pss[0:4, 0:cols], selb[:], y2e[:], start=True, stop=True
                )
                rs = pp.tile([4, cols], F32R, name=names + "r", tag=names + "r")
                with nc.allow_low_precision(reason="f32r view of exact f32 recip"):
                    nc.vector.reciprocal(rs[:], pss[0:4, 0:cols])
                psb = psC.tile([128, 512], F32, tag="c")
                nc.tensor.matmul(
                    psb[0:128, 0:cols], selt[:], rs[:], start=True, stop=True
                )
                t = pp.tile([128, cols], BF16, name=names + "m", tag=names + "m")
                nc.vector.tensor_tensor(
                    t[:], y2e[:], psb[0:128, 0:cols], op=mybir.AluOpType.mult
                )
                return t

            def enc_mm(psenc, ro, wis, off_wi, nmm_last):
                nmm = 0
                for di in range(5):
                    for part in range(2):
                        if part == 0:
                            lhsT = wenc[:, di * 128:di * 128 + 128]
                            kp = 128
                        else:
                            lhsT = wenc[0:32, 640 + di * 128:640 + di * 128 + 128]
                            kp = 32
                        rhs = AP(
                            stk.tensor,
                            (ro + di) * WP + 4 * part + off_wi,
                            [[PADPOS, kp], [1, wis], [2 * WP, 8], [16, 4]],
                        )
                        nc.tensor.matmul(
                            psenc, lhsT, rhs,
                            start=(nmm == 0), stop=(nmm == nmm_last),
                        )
                        nmm += 1

            ebias_sl = {}
            # ro0: full width
            ps0 = psE.tile([128, 512], F32, tag="enc")
            enc_mm(ps0[:], 0, 16, 0, 9)
            ebias_sl["y20"] = ebias[0][:] if with_ebias else None
            yM0 = softmax_norm(ps0[:], 512, "y20")
            ymp0 = pp.tile([25, 2048], BF16, name="yMp0", tag="yMp0")
            for sub in range(4):
                nc.vector.tensor_copy(
                    AP(ymp0.tensor, sub * 32, [[2048, 25], [128, 16], [1, 32]]),
                    AP(yM0.tensor, (32 * sub) * 512, [[512, 25], [32, 16], [1, 32]]),
                )
            # ro1: two wi-halves
            yMp1 = []
            for h in range(2):
                psh = psE.tile([128, 256], F32, tag="enc")
                enc_mm(psh[:], 1, 8, 8 * h, 9)
                ebias_sl[f"y21{h}"] = (
                    ebias[1][:, h * 256:(h + 1) * 256] if with_ebias else None
                )
                ymh = softmax_norm(psh[:], 256, f"y21{h}")
                m1 = pp.tile([25, 1024], BF16, name=f"yMp1{h}", tag=f"yMp1{h}")
                for sub in range(4):
                    nc.vector.tensor_copy(
                        AP(m1.tensor, sub * 32, [[1024, 25], [128, 8], [1, 32]]),
                        AP(ymh.tensor, (32 * sub) * 256, [[256, 25], [32, 8], [1, 32]]),
                    )
                yMp1.append(m1)
            ctx_enc.__exit__(None, None, None)
            ctx_inner.__exit__(None, None, None)
            ctx_mac = tc.tile_pool(name="psM", bufs=4, space="PSUM")
            psM = ctx_mac.__enter__()

            # ---- band build: ybig[:, (ro,wi) blocks] = P_{ro,wi}.T @ yMp
            # views; ro1's copies are split in halves across both engines to
            # cut the latency into the MAC.
            ybig = pp.tile([KDIM, YF], BF16, tag="ybig")
            cp_engs = (nc.vector.tensor_copy, nc.scalar.copy)
            # PSUM sources: only DVE/Act. ro0 copies all on Act (DVE must
            # stay clear for the ro1 softmax chain running concurrently).
            band_engs = (nc.vector.tensor_copy, nc.scalar.copy)
            for ro in range(2):
                for w4 in range(4):
                    ps = psS.tile([128, 512], F32, tag="band")
                    for wq in range(4):
                        wi = w4 * 4 + wq
                        cbase = (ro * 16 + wi) * KDIM
                        ysrc = ymp0[:, wi * 128:(wi + 1) * 128] if ro == 0 else                             yMp1[w4 // 2][:, (wi % 8) * 128:(wi % 8) * 128 + 128]
                        nc.tensor.matmul(
                            ps[0:KDIM, wq * 128:wq * 128 + 128],
                            ppk[:, cbase:cbase + KDIM],
                            ysrc,
                            start=True, stop=True,
                        )
                    col = ro * 2048 + w4 * 512
                    band_engs[w4 % 2](ybig[:, col:col + 512], ps[0:KDIM, :])

            # ---- MAC ----
            osbs = [
                pp.tile([128, 1024], BF16, name=f"osb{i}", tag=f"osb{i}")
                for i in range(8)
            ]
            for g in range(8):
                for ct in range(2):
                    ps = psM.tile([128, 512], F32, tag="mac")
                    for b4 in range(4):
                        tb = g * 4 + b4
                        base = g * 1024 + b4 * 256 + ct * 128
                        nc.tensor.matmul(
                            ps[:, b4 * 128:(b4 + 1) * 128],
                            xcall[:, base:base + 128],
                            AP(ybig.tensor, tb, [[YF, KDIM], [32, 128]]),
                            start=True, stop=True,
                        )
                    q = (g // 2) * 2 + ct
                    cp_engs[(g + ct + 1) % 2](
                        osbs[q][:, (g % 2) * 512:(g % 2) * 512 + 512], ps[:]
                    )
                    if g % 2 == 1:
                        nc.sync.dma_start(
                            out_d[ct, :, (g - 1) * 512:(g + 1) * 512], osbs[q][:]
                        )
            ctx_mac.__exit__(None, None, None)
    nc.compile()
    return nc


_CACHE: dict[bool, object] = {}


def _get_program(with_ebias: bool):
    if with_ebias not in _CACHE:
        _CACHE[with_ebias] = build_program(with_ebias)
    return _CACHE[with_ebias]


def _prep_inputs(x, w_comp, b_comp, w_enc, b_enc):
    """Build the per-core numpy input dicts."""
    from numpy.lib.stride_tricks import sliding_window_view

    x = np.asarray(x, dtype=np.float32)
    w_comp = np.asarray(w_comp, dtype=np.float32)
    b_comp = np.asarray(b_comp, dtype=np.float32)
    w_enc = np.asarray(w_enc, dtype=np.float32)
    b_enc = np.asarray(b_enc, dtype=np.float32)

    # compress weights, channel-tiled: wp128[c', ct*32 + m] = w_comp[m, ct*128+c']
    wp128 = np.zeros((128, 64), dtype=np.float32)
    wp128[:, 0:32] = w_comp.T[0:128]
    wp128[:, 32:64] = w_comp.T[128:256]
    wp128 = wp128.astype(BF16NP)

    # encoder output channel layout: o'' = sub*32 + tap (zeros elsewhere)
    o_src = np.arange(NK)
    o2 = (o_src % 4) * 32 + o_src // 4
    sel = np.zeros((128, 4), dtype=np.float32)
    sel[o2, o_src % 4] = 1.0
    selb = sel.astype(BF16NP)
    selt = np.ascontiguousarray(sel.T)

    # encoder stationaries for the 4-high stacked y1:
    # wenc[32b+m, di*128 + o''] = w_enc[o, m, di, b]; cols 640: hold the
    # K=32 dj=4 slice
    wenc = np.zeros((128, 1280), dtype=np.float32)
    for di in range(5):
        for b in range(4):
            blk = np.zeros((C_MID, 128), dtype=np.float32)
            blk[:, o2] = w_enc[:, :, di, b].T
            wenc[32 * b:32 * b + 32, di * 128:di * 128 + 128] = blk
        blk = np.zeros((C_MID, 128), dtype=np.float32)
        blk[:, o2] = w_enc[:, :, di, 4].T
        wenc[0:32, 640 + di * 128:640 + di * 128 + 128] = blk
    wenc_bf = wenc.astype(BF16NP)

    # band placement matrices P_{ro,wi} [25, 120]
    ppack = np.zeros((25, 32 * KDIM), dtype=np.float32)
    dii = np.repeat(np.arange(5), 5)
    djj = np.tile(np.arange(5), 5)
    for ro in range(2):
        for wi in range(16):
            cols = (ro * 16 + wi) * KDIM + (ro + dii) * 20 + wi + djj
            ppack[np.arange(25), cols] = 1.0
    ppack = ppack.astype(BF16NP)

    with_ebias = bool(b_comp.any() or b_enc.any())

    in_maps = []
    for core in range(NCORES):
        b = core // 4
        h0 = (core % 4) * HSLICE
        xs = np.zeros((C, ROWS, WP), dtype=np.float32)
        r_lo = max(0, h0 - 2)
        r_hi = min(H, h0 + HSLICE + 2)
        xs[:, (r_lo - (h0 - 2)):(r_hi - (h0 - 2)), 2:2 + W] = x[b, :, r_lo:r_hi, :]

        # window-major MAC stationaries:
        # xcall[(r,wc), (g,b4,ct,c')] = xs[ct*128+c', 2g+r, 16b4+wc]
        A = xs.reshape(2, 128, ROWS, WP)
        W4 = sliding_window_view(A, 20, axis=3)          # [2,128,20,49,20]
        Bv = W4[:, :, :, [0, 16, 32, 48], :]             # [2,128,20,4b4,20wc]
        rows = 2 * np.arange(8)[None, :] + np.arange(6)[:, None]  # [6r, 8g]
        Cv = Bv[:, :, rows, :, :]                        # [2,128,6r,8g,4b4,20wc]
        xcall = np.ascontiguousarray(
            Cv.transpose(2, 5, 3, 4, 0, 1)
        ).reshape(KDIM, 8192).astype(BF16NP)

        xinp = np.zeros((2, 128, PADPOS + 1), dtype=BF16NP)
        xinp[:, :, :PADPOS] = xs.reshape(2, 128, PADPOS).astype(BF16NP)
        m = {
            "xin": xinp,
            "xcall": xcall,
            "wp128": wp128,
            "wenc": wenc_bf,
            "selb": selb,
            "selt": selt,
            "ppack": ppack,
        }
        if with_ebias:
            # field[o, h, w] = b_enc[o] + conv of b_comp over the valid mask
            wb = np.einsum("omt,m->ot", we, b_comp).reshape(NK, 5, 5)
            field = np.zeros((NK, HSLICE, W), dtype=np.float32)
            for di in range(-2, 3):
                for dj in range(-2, 3):
                    hh = np.arange(h0, h0 + HSLICE)[:, None] + di
                    ww = np.arange(W)[None, :] + dj
                    valid = ((hh >= 0) & (hh < H) & (ww >= 0) & (ww < W))
                    field += (
                        wb[:, di + 2, dj + 2][:, None, None]
                        * valid[None].astype(np.float32)
                    )
            field += b_enc[:, None, None]
            # columns in (wi, g, b4) order; rows o'' = sub*32 + tap
            f = field.reshape(NK, 8, 2, 4, 16)        # (o, g, ro, b4, wi)
            f = np.transpose(f, (2, 0, 4, 1, 3))      # (ro, o, wi, g, b4)
            f = np.ascontiguousarray(f.reshape(2, NK, 512))
            fe = np.zeros((2, 128, 512), dtype=np.float32)
            fe[:, o2, :] = f
            m["ebias"] = fe
        in_maps.append(m)
    return in_maps, with_ebias


TRACE = False
LAST_RESULT = None


def kernel(x, w_comp, b_comp, w_enc, b_enc):
    global LAST_RESULT
    from concourse.bass_utils import run_bass_kernel_spmd

    in_maps, with_ebias = _prep_inputs(x, w_comp, b_comp, w_enc, b_enc)
    nc = _get_program(with_ebias)
    res = run_bass_kernel_spmd(
        nc, in_maps, core_ids=list(range(NCORES)), trace=TRACE
    )
    LAST_RESULT = res
    out = np.empty((B, C, 2 * H, 2 * W), dtype=np.float32)
    for core in range(NCORES):
        b = core // 4
        h0 = (core % 4) * HSLICE
        o = res.results[core]["out"].astype(np.float32)
        # cols: g*512 + b4*128 + ro*64 + wi*4 + sub; sub = r1*2 + r2
        o = o.reshape(2, 128, 8, 4, 2, 16, 2, 2)
        o = np.transpose(o, (0, 1, 2, 4, 6, 3, 5, 7)).reshape(2, 128, 32, 128)
        out[b, :128, 2 * h0:2 * h0 + 32, :] = o[0]
        out[b, 128:, 2 * h0:2 * h0 + 32, :] = o[1]
    return out



# revision 18
# speedup vs baseline: 1.0167x; 1.0167x over previous
"""CARAFE content-aware upsampling on 8 Trainium2 NeuronCores (Bass/Tile).

Problem: x[2,256,64,64], 1x1 compress conv (256->32), 5x5 encoder conv
(32->100), pixel-shuffle(r=2) + softmax over 25 taps, then dynamic-filter
reassembly: out[b,c,2h+r1,2w+r2] = sum_k x[b,c,h+di,w+dj] * softmax_w.

Sharding: pure data-parallel over (batch, 16-row H slices) -> 8 cores.

Per-core mapping (the cost model charges ~630ns of serialized HWDGE per
DMA and ~1.1us of Pool time per SWDGE DMA, so DMA instruction count is
minimized and all mid-kernel data movement uses engine copies):
  - Host prep ships x twice: channel-major (f32r) for the compress conv
    and window-major xcall [120, 8192] (bf16) holding the overlapping
    6x20 MAC stationaries, so no on-device transpose/gather.
  - compress conv (f32r) writes a 4-high column-shifted stack of y1
    (cross-partition psum copies), so the 5x5 encoder conv contracts 4
    dj taps per K=128 matmul (plus a K=32 dj=4 pass): 10 matmuls per
    row parity instead of 25. Encoder output channels are laid out
    o'' = sub*32 + tap on 128 partitions.
  - softmax stays channel-major (select-matrix matmuls for tap sums and
    reciprocal broadcast); the gating ro=1 parity is split into two
    wi-halves that pipeline through Act/PE/DVE.
  - The normalized weights are relaid out to [25, 2048] (taps on
    partitions) by cross-partition DVE copies, then the block-sparse
    band matrix ybig [120, 4096] (bf16) is built by 128 tiny PE matmuls
    against host-prepared 0/1 placement matrices - the matmul writes the
    band's zeros too, so no memset and no scatter DMAs.
  - The 25-tap dynamic-filter sum runs on PE as 64 bf16 [120]x[128]
    matmuls (stationary = xcall windows, moving = band views).
  - Output is stored bf16 (8 DMAs) and upcast on host; a chain of dummy
    matmuls at t=0 ramps the PE p-state before real work arrives.
"""

import sys

sys.path.insert(0, "/opt/trn_rl_repo")

import numpy as np
import ml_dtypes

import concourse.bacc as bacc
import concourse.bass as bass
import concourse.tile as tile
from concourse import mybir
from concourse.ap import AP

F32 = mybir.dt.float32
F32R = mybir.dt.float32r
BF16 = mybir.dt.bfloat16
BF16NP = ml_dtypes.bfloat16

# geometry
B, C, H, W = 2, 256, 64, 64
RATIO, K_UP, C_MID, ENC_K = 2, 5, 32, 5
NK = RATIO * RATIO * K_UP * K_UP  # 100
HSLICE = 16                       # output source rows per core
ROWS = HSLICE + 4                 # with 2-row halo each side
WP = W + 4                        # padded width
PADPOS = ROWS * WP                # 1360
NCORES = 8
KDIM = 120                        # 6x20 window pixels per row-pair block
YF = 4096                         # band matrix columns
NPRIME = 34                       # PE p-state priming matmuls


def build_program(with_ebias: bool):
    nc = bacc.Bacc()
    xin_d = nc.declare_dram_parameter("xin", [2, 128, PADPOS + 1], BF16, isOutput=False)
    xc_d = nc.declare_dram_parameter("xcall", [KDIM, 8192], BF16, isOutput=False)
    wp_d = nc.declare_dram_parameter("wp128", [128, 64], BF16, isOutput=False)
    wet_d = nc.declare_dram_parameter("wenc", [128, 1280], BF16, isOutput=False)
    selb_d = nc.declare_dram_parameter("selb", [128, 4], BF16, isOutput=False)
    selt_d = nc.declare_dram_parameter("selt", [4, 128], F32R, isOutput=False)
    pp_d = nc.declare_dram_parameter("ppack", [25, 32 * KDIM], BF16, isOutput=False)
    if with_ebias:
        ebias_d = nc.declare_dram_parameter("ebias", [2, 128, 512], F32, isOutput=False)
    out_d = nc.declare_dram_parameter("out", [2, 128, YF], BF16, isOutput=True)

    with tile.TileContext(nc) as tc:
        # Partition-crossing DMA APs (relayout) confuse the byte-range race
        # detector; deps are tracked at tensor granularity regardless.
        tc.race_detector_enabled = False
        # PSUM is 8 banks x 2KB/partition; pools cost bufs x (bank-rounded
        # slot per tag). psC/psE are scoped to the conv/softmax phase and
        # released before the MAC pool opens: 4+2+2 banks early, 4+4 late.
        with (
            tc.tile_pool(name="persist", bufs=1) as pp,
            tc.tile_pool(name="psS", bufs=4, space="PSUM") as psS,   # prime/band
        ):
            # ---- PE p-state priming: keep PE busy from t=0 so real matmuls
            # run at the full-ramp cycle time when inputs arrive.
            dummy = pp.tile([128, 128], BF16, tag="dummy")
            nc.vector.memset(dummy[:], 0.0)
            for _ in range(NPRIME):
                ps = psS.tile([128, 512], F32, tag="band")
                nc.tensor.matmul(
                    ps[:, 0:128], dummy[:], dummy[:], start=True, stop=True
                )

            # ---- input loads ----
            # Every load rides the SP HWDGE queue in strict need-order: the
            # single HWDGE device serializes descriptor-gen (~630ns each), so
            # one queue in need-order makes wire order deterministic and keeps
            # the Pool engine free of SWDGE work. xin ships in 2 chunks per
            # channel-tile so compress starts as soon as rows 0-11 land.
            wp128 = pp.tile([128, 64], BF16, tag="wp128")
            nc.sync.dma_start(wp128[:], wp_d[:])
            xin0 = pp.tile([128, PADPOS + 1], BF16, tag="xin0")
            xin1 = pp.tile([128, PADPOS + 1], BF16, tag="xin1")
            # compress chunks sized so chunk 1's +1-shifted read stays inside
            # dma chunk 0 (cols 0-815): psum-bank limit keeps each <= 512
            chunks = [(0, 407), (407, 407), (814, 273), (1087, PADPOS - 1087)]
            # dma chunks: rows 0-11 (816 cols), rows 12-19 + pad col (545)
            XSPLIT = 816
            for t, d in ((xin0, xin_d[0]), (xin1, xin_d[1])):
                nc.sync.dma_start(t[:, 0:XSPLIT], d[:, 0:XSPLIT])
            for t, d in ((xin0, xin_d[0]), (xin1, xin_d[1])):
                nc.sync.dma_start(t[:, XSPLIT:], d[:, XSPLIT:])
            wenc = pp.tile([128, 1280], BF16, tag="wenc")
            nc.sync.dma_start(wenc[:], wet_d[:])
            selb = pp.tile([128, 4], BF16, tag="selb")
            nc.sync.dma_start(selb[:], selb_d[:])
            selt = pp.tile([4, 128], F32R, tag="selt")
            nc.sync.dma_start(selt[:], selt_d[:])
            ppk = pp.tile([25, 32 * KDIM], BF16, tag="ppack")
            nc.sync.dma_start(ppk[:], pp_d[:])
            if with_ebias:
                ebias = []
                for ro in range(2):
                    t = pp.tile([128, 512], F32, name=f"ebias{ro}", tag=f"ebias{ro}")
                    nc.sync.dma_start(t[:], ebias_d[ro])
                    ebias.append(t)
            xcall = pp.tile([KDIM, 8192], BF16, tag="xcall")
            nc.sync.dma_start(xcall[:, 0:4096], xc_d[:, 0:4096])
            nc.sync.dma_start(xcall[:, 4096:], xc_d[:, 4096:])

            # ---- compress conv -> stacked [128, PADPOS]: row block 32b
            # holds y1[m, p+b]. Blocks 0/1 come from matmul pairs against
            # col-shifted x; blocks 2/3 are chunk-aligned shifted copies of
            # blocks 0/1, so the encoder contracts four dj taps per matmul
            # with K=128 at base partition 0.
            stk = pp.tile([128, PADPOS], BF16, tag="stk")
            ctx_inner = tc.tile_pool(name="psC", bufs=2, space="PSUM")
            psC = ctx_inner.__enter__()
            ctx_enc = tc.tile_pool(name="psE", bufs=2, space="PSUM")
            psE = ctx_enc.__enter__()
            for ci, (off, n) in enumerate(chunks):
                for b in range(2):
                    ps = psC.tile([128, 512], F32, tag="c")
                    nc.tensor.matmul(
                        ps[0:C_MID, :n],
                        wp128[:, 0:32], xin0[:, off + b:off + b + n],
                        start=True, stop=False,
                    )
                    nc.tensor.matmul(
                        ps[0:C_MID, :n],
                        wp128[:, 32:64], xin1[:, off + b:off + b + n],
                        start=False, stop=True,
                    )
                    # cross-partition copy drops the shifted rows into the
                    # stack's b-th 32-row block (PSUM source: only DVE/Act
                    # may read PSUM; GPSIMD cannot).
                    eng = (nc.vector.tensor_copy, nc.scalar.copy)[b]
                    eng(stk[32 * b:32 * b + 32, off:off + n], ps[0:C_MID, :n])
                # blocks 2/3: +2-shifted copies of blocks 0/1, chunk-aligned
                # (chunk i's source columns live in chunks i and i+1, so
                # shift the window 2 left to stay within loaded data)
                s0 = max(0, off - 2)
                s1 = off + n - 2
                # SBUF->SBUF bf16: Pool is free (no SWDGE anymore), keep
                # DVE/Act clear for the PSUM-source copies
                nc.gpsimd.tensor_copy(stk[64:128, s0:s1], stk[0:64, s0 + 2:s1 + 2])
                if ci == len(chunks) - 1:
                    # last two columns of blocks 2/3 (never read, but keep
                    # them initialized for the simulator)
                    nc.gpsimd.tensor_copy(
                        stk[64:128, s1:s1 + 2], stk[0:64, s1:s1 + 2]
                    )

            # ---- encoder conv + softmax ----
            # ro0's softmax chain is hidden under ro1's encoder; ro1's chain
            # gates the MAC, so it is split into two wi-halves that pipeline
            # through PE/Act/DVE, letting its band build start ~2.5us earlier.
            yM = []
            yMp = []

            def softmax_norm(psenc, cols, names):
                # returns normalized bf16 weights tile [128, cols]
                y2e = pp.tile([128, cols], BF16, name=names + "e", tag=names + "e")
                if with_ebias:
                    y2f = pp.tile([128, cols], F32, name=names + "f", tag=names + "f")
                    nc.vector.scalar_tensor_tensor(
                        y2f[:], psenc, 1.0, ebias_sl[names],
                        op0=mybir.AluOpType.mult, op1=mybir.AluOpType.add,
                    )
                    nc.scalar.activation(
                        y2e[:], y2f[:], mybir.ActivationFunctionType.Exp
                    )
                else:
                    nc.scalar.activation(
                        y2e[:], psenc, mybir.ActivationFunctionType.Exp
                    )
                pss = psC.tile([128, 512], F32, tag="c")
                nc.tensor.matmul(
                    pss[0:4, 0:cols], selb[:], y2e[:], start=True, stop=True
                )
                rs = pp.tile([4, cols], F32R, name=names + "r", tag=names + "r")
                with nc.allow_low_precision(reason="f32r view of exact f32 recip"):
                    nc.vector.reciprocal(rs[:], pss[0:4, 0:cols])
                psb = psC.tile([128, 512], F32, tag="c")
                nc.tensor.matmul(
                    psb[0:128, 0:cols], selt[:], rs[:], start=True, stop=True
                )
                t = pp.tile([128, cols], BF16, name=names + "m", tag=names + "m")
                nc.vector.tensor_tensor(
                    t[:], y2e[:], psb[0:128, 0:cols], op=mybir.AluOpType.mult
                )
                return t

            def enc_mm(psenc, ro, wis, off_wi, nmm_last):
                nmm = 0
                for di in range(5):
                    for part in range(2):
                        if part == 0:
                            lhsT = wenc[:, di * 128:di * 128 + 128]
                            kp = 128
                        else:
                            lhsT = wenc[0:32, 640 + di * 128:640 + di * 128 + 128]
                            kp = 32
                        rhs = AP(
                            stk.tensor,
                            (ro + di) * WP + 4 * part + off_wi,
                            [[PADPOS, kp], [1, wis], [2 * WP, 8], [16, 4]],
                        )
                        nc.tensor.matmul(
                            psenc, lhsT, rhs,
                            start=(nmm == 0), stop=(nmm == nmm_last),
                        )
                        nmm += 1

            ebias_sl = {}
            # ro0: full width
            ps0 = psE.tile([128, 512], F32, tag="enc")
            enc_mm(ps0[:], 0, 16, 0, 9)
            ebias_sl["y20"] = ebias[0][:] if with_ebias else None
            yM0 = softmax_norm(ps0[:], 512, "y20")
            ymp0 = pp.tile([25, 2048], BF16, name="yMp0", tag="yMp0")
            for sub in range(4):
                nc.vector.tensor_copy(
                    AP(ymp0.tensor, sub * 32, [[2048, 25], [128, 16], [1, 32]]),
                    AP(yM0.tensor, (32 * sub) * 512, [[512, 25], [32, 16], [1, 32]]),
                )
            # ro1: two wi-halves
            yMp1 = []
            for h in range(2):
                psh = psE.tile([128, 256], F32, tag="enc")
                enc_mm(psh[:], 1, 8, 8 * h, 9)
                ebias_sl[f"y21{h}"] = (
                    ebias[1][:, h * 256:(h + 1) * 256] if with_ebias else None
                )
                ymh = softmax_norm(psh[:], 256, f"y21{h}")
                m1 = pp.tile([25, 1024], BF16, name=f"yMp1{h}", tag=f"yMp1{h}")
                for sub in range(4):
                    nc.vector.tensor_copy(
                        AP(m1.tensor, sub * 32, [[1024, 25], [128, 8], [1, 32]]),
                        AP(ymh.tensor, (32 * sub) * 256, [[256, 25], [32, 8], [1, 32]]),
                    )
                yMp1.append(m1)
            ctx_enc.__exit__(None, None, None)
            ctx_inner.__exit__(None, None, None)
            ctx_mac = tc.tile_pool(name="psM", bufs=4, space="PSUM")
            psM = ctx_mac.__enter__()

            # ---- band build: ybig[:, (ro,wi) blocks] = P_{ro,wi}.T @ yMp
            # views; ro1's copies are split in halves across both engines to
            # cut the latency into the MAC.
            ybig = pp.tile([KDIM, YF], BF16, tag="ybig")
            cp_engs = (nc.vector.tensor_copy, nc.scalar.copy)
            # PSUM sources: only DVE/Act. ro0 copies all on Act (DVE must
            # stay clear for the ro1 softmax chain running concurrently).
            band_engs = (nc.vector.tensor_copy, nc.scalar.copy)
            for ro in range(2):
                for w4 in range(4):
                    ps = psS.tile([128, 512], F32, tag="band")
                    for wq in range(4):
                        wi = w4 * 4 + wq
                        cbase = (ro * 16 + wi) * KDIM
                        ysrc = ymp0[:, wi * 128:(wi + 1) * 128] if ro == 0 else                             yMp1[w4 // 2][:, (wi % 8) * 128:(wi % 8) * 128 + 128]
                        nc.tensor.matmul(
                            ps[0:KDIM, wq * 128:wq * 128 + 128],
                            ppk[:, cbase:cbase + KDIM],
                            ysrc,
                            start=True, stop=True,
                        )
                    col = ro * 2048 + w4 * 512
                    band_engs[w4 % 2](ybig[:, col:col + 512], ps[0:KDIM, :])

            # ---- MAC ----
            osbs = [
                pp.tile([128, 1024], BF16, name=f"osb{i}", tag=f"osb{i}")
                for i in range(8)
            ]
            for g in range(8):
                for ct in range(2):
                    ps = psM.tile([128, 512], F32, tag="mac")
                    for b4 in range(4):
                        tb = g * 4 + b4
                        base = g * 1024 + b4 * 256 + ct * 128
                        nc.tensor.matmul(
                            ps[:, b4 * 128:(b4 + 1) * 128],
                            xcall[:, base:base + 128],
                            AP(ybig.tensor, tb, [[YF, KDIM], [32, 128]]),
                            start=True, stop=True,
                        )
                    q = (g // 2) * 2 + ct
                    cp_engs[(g + ct + 1) % 2](
                        osbs[q][:, (g % 2) * 512:(g % 2) * 512 + 512], ps[:]
                    )
                    if g % 2 == 1:
                        nc.sync.dma_start(
                            out_d[ct, :, (g - 1) * 512:(g + 1) * 512], osbs[q][:]
                        )
            ctx_mac.__exit__(None, None, None)
    nc.compile()
    return nc


_CACHE: dict[bool, object] = {}


def _get_program(with_ebias: bool):
    if with_ebias not in _CACHE:
        _CACHE[with_ebias] = build_program(with_ebias)
    return _CACHE[with_ebias]


def _prep_inputs(x, w_comp, b_comp, w_enc, b_enc):
    """Build the per-core numpy input dicts."""
    from numpy.lib.stride_tricks import sliding_window_view

    x = np.asarray(x, dtype=np.float32)
    w_comp = np.asarray(w_comp, dtype=np.float32)
    b_comp = np.asarray(b_comp, dtype=np.float32)
    w_enc = np.asarray(w_enc, dtype=np.float32)
    b_enc = np.asarray(b_enc, dtype=np.float32)

    # compress weights, channel-tiled: wp128[c', ct*32 + m] = w_comp[m, ct*128+c']
    wp128 = np.zeros((128, 64), dtype=np.float32)
    wp128[:, 0:32] = w_comp.T[0:128]
    wp128[:, 32:64] = w_comp.T[128:256]
    wp128 = wp128.astype(BF16NP)

    # encoder output channel layout: o'' = sub*32 + tap (zeros elsewhere)
    o_src = np.arange(NK)
    o2 = (o_src % 4) * 32 + o_src // 4
    sel = np.zeros((128, 4), dtype=np.float32)
    sel[o2, o_src % 4] = 1.0
    selb = sel.astype(BF16NP)
    selt = np.ascontiguousarray(sel.T)

    # encoder stationaries for the 4-high stacked y1:
    # wenc[32b+m, di*128 + o''] = w_enc[o, m, di, b]; cols 640: hold the
    # K=32 dj=4 slice
    wenc = np.zeros((128, 1280), dtype=np.float32)
    for di in range(5):
        for b in range(4):
            blk = np.zeros((C_MID, 128), dtype=np.float32)
            blk[:, o2] = w_enc[:, :, di, b].T
            wenc[32 * b:32 * b + 32, di * 128:di * 128 + 128] = blk
        blk = np.zeros((C_MID, 128), dtype=np.float32)
        blk[:, o2] = w_enc[:, :, di, 4].T
        wenc[0:32, 640 + di * 128:640 + di * 128 + 128] = blk
    wenc_bf = wenc.astype(BF16NP)

    # band placement matrices P_{ro,wi} [25, 120]
    ppack = np.zeros((25, 32 * KDIM), dtype=np.float32)
    dii = np.repeat(np.arange(5), 5)
    djj = np.tile(np.arange(5), 5)
    for ro in range(2):
        for wi in range(16):
            cols = (ro * 16 + wi) * KDIM + (ro + dii) * 20 + wi + djj
            ppack[np.arange(25), cols] = 1.0
    ppack = ppack.astype(BF16NP)

    with_ebias = bool(b_comp.any() or b_enc.any())

    in_maps = []
    for core in range(NCORES):
        b = core // 4
        h0 = (core % 4) * HSLICE
        xs = np.zeros((C, ROWS, WP), dtype=np.float32)
        r_lo = max(0, h0 - 2)
        r_hi = min(H, h0 + HSLICE + 2)
        xs[:, (r_lo - (h0 - 2)):(r_hi - (h0 - 2)), 2:2 + W] = x[b, :, r_lo:r_hi, :]

        # window-major MAC stationaries:
        # xcall[(r,wc), (g,b4,ct,c')] = xs[ct*128+c', 2g+r, 16b4+wc]
        A = xs.reshape(2, 128, ROWS, WP)
        W4 = sliding_window_view(A, 20, axis=3)          # [2,128,20,49,20]
        Bv = W4[:, :, :, [0, 16, 32, 48], :]             # [2,128,20,4b4,20wc]
        rows = 2 * np.arange(8)[None, :] + np.arange(6)[:, None]  # [6r, 8g]
        Cv = Bv[:, :, rows, :, :]                        # [2,128,6r,8g,4b4,20wc]
        xcall = np.ascontiguousarray(
            Cv.transpose(2, 5, 3, 4, 0, 1)
        ).reshape(KDIM, 8192).astype(BF16NP)

        xinp = np.zeros((2, 128, PADPOS + 1), dtype=BF16NP)
        xinp[:, :, :PADPOS] = xs.reshape(2, 128, PADPOS).astype(BF16NP)
        m = {
            "xin": xinp,
            "xcall": xcall,
            "wp128": wp128,
            "wenc": wenc_bf,
            "selb": selb,
            "selt": selt,
            "ppack": ppack,
        }
        if with_ebias:
            # field[o, h, w] = b_enc[o] + conv of b_comp over the valid mask
            wb = np.einsum("omt,m->ot", we, b_comp).reshape(NK, 5, 5)
            field = np.zeros((NK, HSLICE, W), dtype=np.float32)
            for di in range(-2, 3):
                for dj in range(-2, 3):
                    hh = np.arange(h0, h0 + HSLICE)[:, None] + di
                    ww = np.arange(W)[None, :] + dj
                    valid = ((hh >= 0) & (hh < H) & (ww >= 0) & (ww < W))
                    field += (
                        wb[:, di + 2, dj + 2][:, None, None]
                        * valid[None].astype(np.float32)
                    )
            field += b_enc[:, None, None]
            # columns in (wi, g, b4) order; rows o'' = sub*32 + tap
            f = field.reshape(NK, 8, 2, 4, 16)        # (o, g, ro, b4, wi)
            f = np.transpose(f, (2, 0, 4, 1, 3))      # (ro, o, wi, g, b4)
            f = np.ascontiguousarray(f.reshape(2, NK, 512))
            fe = np.zeros((2, 128, 512), dtype=np.float32)
            fe[:, o2, :] = f
            m["ebias"] = fe
        in_maps.append(m)
    return in_maps, with_ebias


TRACE = False
LAST_RESULT = None


def kernel(x, w_comp, b_comp, w_enc, b_enc):
    global LAST_RESULT
    from concourse.bass_utils import run_bass_kernel_spmd

    in_maps, with_ebias = _prep_inputs(x, w_comp, b_comp, w_enc, b_enc)
    nc = _get_program(with_ebias)
    res = run_bass_kernel_spmd(
        nc, in_maps, core_ids=list(range(NCORES)), trace=TRACE
    )
    LAST_RESULT = res
    out = np.empty((B, C, 2 * H, 2 * W), dtype=np.float32)
    for core in range(NCORES):
        b = core // 4
        h0 = (core % 4) * HSLICE
        o = res.results[core]["out"].astype(np.float32)
        # cols: g*512 + b4*128 + ro*64 + wi*4 + sub; sub = r1*2 + r2
        o = o.reshape(2, 128, 8, 4, 2, 16, 2, 2)
        o = np.transpose(o, (0, 1, 2, 4, 6, 3, 5, 7)).reshape(2, 128, 32, 128)
        out[b, :128, 2 * h0:2 * h0 + 32, :] = o[0]
        out[b, 128:, 2 * h0:2 * h0 + 32, :] = o[1]
    return out

